# revision 1
# baseline (speedup 1.0000x reference)
"""Trainium2 Bass kernel for the e3nn-style 3D convolution problem.

Host side: builds the tiny [3,3,3,32,64] conv kernel from the radial/spherical
weights (replicating the reference math in fp32 numpy), folds the pointwise
self-connection into the center tap, and pre-arranges the input as a z-im2col
(3 z-shifted copies x 32 channels = 96 partitions) padded volume per batch.

Device side (per core, batch-parallel over 8 cores): 3D conv as accumulated
fp32r matmuls with contraction K=96 (3 z-taps x 32 ch) and dx-pairing in the
stationary operand (M=128 = [W(dx=-1) | W(dx=+1)]), PSUM accumulation with
cross-plane deferred evacuation, output staged as [128, 512] per-plane tiles
(both y-halves) for full-port DMA out.
"""

import math

import numpy as np

# ---- problem constants (hardcoded; kernel.py must be self-contained) ----
MUL_IN, MUL_OUT = 8, 16
DIM_IN, DIM_OUT = 4 * MUL_IN, 4 * MUL_OUT  # 32, 64
DIAMETER = 3.0
NUM_RB = 4
BATCH, GRID = 8, 32
N_CORES = 8

XP = GRID + 2  # padded x planes: -1 .. 32
YP = GRID + 2  # padded y rows
PLANE = YP * GRID  # floats per (padded-y, z) plane = 34*32 = 1088
XFREE = XP * PLANE  # per-partition floats of the im2col tile = 36992
KPART = 3 * DIM_IN  # 96 partitions: z-shift blocks (dz=-1,0,+1) x 32 channels
OUT_COLS = GRID * 512  # out dram [128, 16384]

# matmul operand dtype: "fp32r" = full PE rate with ~1.9e-4 relative error,
# "fp32" = exact but 1/4 PE rate.
MM_DTYPE = "fp32r"


# --------------------------------------------------------------------------
# host-side math: replicate the reference kernel build in fp32 numpy
# --------------------------------------------------------------------------
def _sus(x):
    # smooth unit step: exp(-1/x) for x>0 else 0
    safe = np.where(x > 0.0, x, 1.0).astype(np.float32)
    return np.where(x > 0.0, np.exp(np.float32(-1.0) / safe), np.float32(0.0))


def build_conv_kernel(w_lin0, w_lin1, w000, w011, w101, w110):
    """Returns K [3,3,3,DIM_IN,DIM_OUT] fp32 with the self-connection folded
    into the center tap."""
    f32 = np.float32
    r = DIAMETER / 2
    ax = np.arange(-math.floor(r), math.floor(r) + 1.0, dtype=f32)  # [-1,0,1]
    lattice = np.stack(np.meshgrid(ax, ax, ax, indexing="ij"), axis=-1).astype(f32)

    dist = np.linalg.norm(lattice, axis=-1).astype(f32)  # [3,3,3]
    values = np.linspace(0.0, DIAMETER / 2, NUM_RB + 2, dtype=f32)
    step = values[1] - values[0]
    diff = (dist[..., None] - values[1:-1]) / step  # [3,3,3,4]
    emb = (f32(1.14136) * np.exp(f32(2.0)) * _sus(diff + 1.0) * _sus(1.0 - diff)).astype(f32)

    norm = np.linalg.norm(lattice, axis=-1, keepdims=True).astype(f32)
    unit = lattice / np.where(norm == 0.0, f32(1.0), norm)
    sh1 = (np.sqrt(f32(3.0)) * unit).astype(f32)  # [3,3,3,3]

    n_lat = 27

    def rad(w):
        # emb [3,3,3,4] x w [4,8,1,16] -> [3,3,3,8,16]
        return (np.einsum("xyzk,kuvw->xyzuw", emb, w.astype(f32)) / f32(n_lat)).astype(f32)

    r000, r011, r101, r110 = rad(w000), rad(w011), rad(w101), rad(w110)

    inv_s3 = f32(1.0 / math.sqrt(3.0))
    alpha = f32(1.0 / math.sqrt(2.0 * MUL_IN))

    k00 = (alpha * r000).astype(f32)  # [3,3,3,8,16]
    k01 = (alpha * inv_s3) * np.einsum("xyzuw,xyzm->xyzuwm", r011, sh1)
    k01 = k01.reshape(3, 3, 3, MUL_IN, 3 * MUL_OUT).astype(f32)
    k10 = (alpha * inv_s3) * np.einsum("xyzuw,xyzi->xyzuiw", r110, sh1)
    k10 = k10.reshape(3, 3, 3, 3 * MUL_IN, MUL_OUT).astype(f32)
    eye3 = np.eye(3, dtype=f32)
    k11 = (alpha * inv_s3) * np.einsum("xyzuw,im->xyzuiwm", r101, eye3)
    k11 = k11.reshape(3, 3, 3, 3 * MUL_IN, 3 * MUL_OUT).astype(f32)

    k = np.concatenate(
        [
            np.concatenate([k00, k01], axis=-1),
            np.concatenate([k10, k11], axis=-1),
        ],
        axis=-2,
    ).astype(f32)  # [3,3,3,32,64]

    # ---- self-connection folded into the center tap ----
    lin_norm = f32(1.0 / math.sqrt(MUL_IN))
    w_sc = np.zeros((DIM_IN, DIM_OUT), f32)
    w_sc[:MUL_IN, :MUL_OUT] = w_lin0.astype(f32) * lin_norm
    for i in range(3):
        rows = MUL_IN + 3 * np.arange(MUL_IN) + i
        cols = MUL_OUT + 3 * np.arange(MUL_OUT) + i
        w_sc[np.ix_(rows, cols)] = w_lin1.astype(f32) * lin_norm
    k[1, 1, 1] += w_sc
    return k


def pack_weights(k):
    """[3,3,3,32,64] -> wk [96, 576] in the dx-paired layout:

    cols [128*ky, 128*ky+64)       = tap (kx=0, ky)  rows 32*kz+ci
    cols [128*ky+64, 128*ky+128)   = tap (kx=2, ky)
    cols [384+64*ky, 384+64*ky+64) = tap (kx=1, ky)  ("singles")
    """
    wk = np.zeros((KPART, 9 * DIM_OUT), np.float32)
    for ky in range(3):
        wk[:, 128 * ky : 128 * ky + 64] = k[0, ky].reshape(KPART, DIM_OUT)
        wk[:, 128 * ky + 64 : 128 * ky + 128] = k[2, ky].reshape(KPART, DIM_OUT)
        wk[:, 384 + 64 * ky : 384 + 64 * (ky + 1)] = k[1, ky].reshape(KPART, DIM_OUT)
    return wk


def build_im2col(xb):
    """xb [32,32,32,32] (X,Y,Z,C) -> xim [96, XFREE] fp32.

    Partition 32*j + c holds x[., ., z + (j-1), c] laid out as
    [xp 0..33][yp 0..33][z 0..31] with zero padding at xp/yp borders and
    z-shift edges."""
    xt = np.ascontiguousarray(xb.transpose(3, 0, 1, 2))  # [C, X, Y, Z]
    xim = np.zeros((KPART, XP, YP, GRID), np.float32)
    xim[0:32, 1:33, 1:33, 1:32] = xt[:, :, :, 0:31]  # dz=-1
    xim[32:64, 1:33, 1:33, :] = xt  # dz=0
    xim[64:96, 1:33, 1:33, 0:31] = xt[:, :, :, 1:32]  # dz=+1
    return xim.reshape(KPART, XFREE)


def gather_out(arr):
    """arr [128, 16384] -> [32, 32, 32, 64].

    Row p = (h*64 + co); column = xi*512 + yi*32 + z."""
    a = arr.reshape(2, DIM_OUT, GRID, 16, GRID)  # [h, co, xi, yi, z]
    return np.ascontiguousarray(a.transpose(2, 0, 3, 4, 1)).reshape(GRID, GRID, GRID, DIM_OUT)


# --------------------------------------------------------------------------
# device program
# --------------------------------------------------------------------------
_PROGRAM_CACHE = {}


def _mm_dt(mybir):
    return mybir.dt.float32r if MM_DTYPE == "fp32r" else mybir.dt.float32


def build_program():
    import concourse.mybir as mybir
    import concourse.tile as tile
    from concourse import bacc

    nc = bacc.Bacc(
        "TRN2",
        target_bir_lowering=False,
        debug=False,
        enable_asserts=True,
        num_devices=N_CORES,
    )
    mdt = _mm_dt(mybir)
    xim_d = nc.dram_tensor("xim", [KPART, XFREE], mdt, kind="ExternalInput").ap()
    wk_d = nc.dram_tensor("wk", [KPART, 9 * DIM_OUT], mdt, kind="ExternalInput").ap()
    out_d = nc.dram_tensor("out", [2 * DIM_OUT, OUT_COLS], mybir.dt.float32, kind="ExternalOutput").ap()

    with tile.TileContext(nc) as tc:
        emit_body(nc, tc, xim_d, wk_d, out_d)

    nc.compile()
    return nc


def emit_body(nc, tc, xim_d, wk_d, out_d, mode="full"):
    """dx-paired scheme with per-plane [128, 512] output staging.

    For out-plane group xi (streaming base plane xp=xi, i.e. x[xi-1]):
      psum rows 0-63   accumulate taps (kx=0, ky) + (kx=1, ky) for out plane xi
      psum rows 64-127 accumulate taps (kx=2, ky) for out plane xi-2
    Evacuation of plane xi: ob[h*64:(h+1)*64] = bank[xi,h][0:64]
    (+ bank[xi+2,h][64:128]), then one [128, 512] DMA per plane.
    """
    import concourse.mybir as mybir

    f32 = mybir.dt.float32
    mdt = _mm_dt(mybir)

    do_in = mode not in ("noin", "mmpure")
    do_mm = mode != "dma"
    do_evac = mode in ("full", "noin", "dma")

    IN_CHUNKS = globals().get("IN_CHUNKS_OVR", 8)
    ob_bufs = globals().get("OB_BUFS_OVR", 6)
    out_eng = getattr(nc, globals().get("OUT_ENGINE", "scalar"))
    in_eng = getattr(nc, globals().get("IN_ENGINE", "sync"))
    evac_split = globals().get("EVAC_SPLIT", True)

    with (
        tc.tile_pool(name="xim", bufs=1) as xim_pool,
        tc.tile_pool(name="wk", bufs=1) as wk_pool,
        tc.tile_pool(name="ob", bufs=ob_bufs) as ob_pool,
        tc.tile_pool(name="ps", bufs=8, space="PSUM") as ps_pool,
    ):
        wk_t = wk_pool.tile([KPART, 9 * DIM_OUT], mdt)
        nc.sync.dma_start(out=wk_t[:, :], in_=wk_d[:, :])

        xim_t = xim_pool.tile([KPART, XFREE], mdt)
        if do_in:
            # chunked so matmuls can start once their planes have landed
            # (Tile tracks subtile deps)
            rows = XFREE // IN_CHUNKS
            for ci in range(IN_CHUNKS):
                lo = ci * rows
                hi = XFREE if ci == IN_CHUNKS - 1 else (ci + 1) * rows
                in_eng.dma_start(out=xim_t[:, lo:hi], in_=xim_d[:, lo:hi])
        else:
            nc.sync.dma_start(out=xim_t[:, 0:128], in_=xim_d[:, 0:128])

        if mode == "dma":
            ob0 = ob_pool.tile([2 * DIM_OUT, 512], f32)
            nc.vector.memset(ob0[:, :], 0.0)
            for xi in range(GRID):
                out_eng.dma_start(
                    out=out_d[:, xi * 512 : (xi + 1) * 512], in_=ob0[:, :]
                )
            return

        def rhs_slice(xp, y):
            off = xp * PLANE + y * GRID
            return xim_t[:, off : off + 512]

        banks = {}
        obs = {}

        def evac(xi, h):
            if not do_evac:
                del banks[(xi, h)]
                return
            if xi not in obs:
                obs[xi] = ob_pool.tile([2 * DIM_OUT, 512], f32, name=f"ob_{xi}", tag="ob")
            ob = obs[xi][h * DIM_OUT : (h + 1) * DIM_OUT, :]
            pa = banks[(xi, h)]
            if evac_split:
                nc.scalar.copy(ob[:, :], pa[0:DIM_OUT, :])
            else:
                nc.vector.tensor_copy(ob[:, :], pa[0:DIM_OUT, :])
            if xi < GRID - 1:
                pb = banks[(xi + 2, h)]
                nc.vector.tensor_add(ob[:, :], ob[:, :], pb[DIM_OUT : 2 * DIM_OUT, :])
            del banks[(xi, h)]
            if h == 1:
                out_eng.dma_start(
                    out=out_d[:, xi * 512 : (xi + 1) * 512], in_=obs[xi][:, :]
                )
                del obs[xi]

        # groups xi = 0..32; group 32 runs pairs only (feeds out plane 30)
        for blk in range(GRID // 2 + 1):  # blocks of up to 2 plane-groups
            gxs = [g for g in (2 * blk, 2 * blk + 1) if g <= GRID]
            for g in gxs:
                for h in (0, 1):
                    banks[(g, h)] = ps_pool.tile(
                        [2 * DIM_OUT, 512], f32, name=f"bank_{g}_{h}", tag="bank"
                    )
            # weight-major inner order: each stationary loaded once per block.
            # Order per bank: pair ky=0 (start=True, writes full rows 0-127)
            # -> singles (rows 0-63) -> pair ky=1 -> pair ky=2 (stop=True).
            # The LAST matmul of each bank is a full-region pair so any PSUM
            # read of rows 64-127 depends on the bank's final matmul —
            # otherwise DVE evac reads race later PE writes to the same bank
            # (fatal PSUM collision on HW).
            # w -> (kind, ky): 0: pair0, 1-3: singles 0-2, 4: pair1, 5: pair2
            for w in range(6):
                for xi in gxs:
                    if not do_mm or (xi == GRID and w in (1, 2, 3)):
                        continue  # group 32: pairs only
                    for h in (0, 1):
                        y0 = h * 16
                        ps = banks[(xi, h)]
                        if w in (0, 4, 5):  # pair ky, base plane xp=xi
                            ky = {0: 0, 4: 1, 5: 2}[w]
                            nc.tensor.matmul(
                                out=ps[:, :],
                                lhsT=wk_t[:, 128 * ky : 128 * (ky + 1)],
                                rhs=rhs_slice(xi, y0 + ky),
                                start=(w == 0),
                                stop=(w == 5),
                            )
                        else:  # single ky=w-1, base plane xp=xi+1
                            ky = w - 1
                            nc.tensor.matmul(
                                out=ps[0:DIM_OUT, :],
                                lhsT=wk_t[:, 384 + 64 * ky : 384 + 64 * (ky + 1)],
                                rhs=rhs_slice(xi + 1, y0 + ky),
                                start=False,
                                stop=False,
                            )
            # planes 2*blk-2 and 2*blk-1 are now complete
            for g in gxs:
                xr = g - 2
                if 0 <= xr < GRID:
                    for h in (0, 1):
                        evac(xr, h)
        for h in (0, 1):
            evac(GRID - 1, h)
            del banks[(GRID, h)]
        assert not banks, f"unevacuated banks: {list(banks)}"


# --------------------------------------------------------------------------
# runner
# --------------------------------------------------------------------------
def _get_program():
    if "nc" not in _PROGRAM_CACHE:
        _PROGRAM_CACHE["nc"] = build_program()
    return _PROGRAM_CACHE["nc"]


def kernel(x, w_lin0, w_lin1, w000, w011, w101, w110):
    from concourse.bass_utils import run_bass_kernel_spmd

    x = np.asarray(x, np.float32)
    k = build_conv_kernel(
        np.asarray(w_lin0), np.asarray(w_lin1),
        np.asarray(w000), np.asarray(w011), np.asarray(w101), np.asarray(w110),
    )
    wk = pack_weights(k)

    in_maps = [{"xim": build_im2col(x[b]), "wk": wk} for b in range(BATCH)]

    nc = _get_program()
    res = run_bass_kernel_spmd(nc, in_maps, list(range(N_CORES)))

    out = np.empty((BATCH, GRID, GRID, GRID, DIM_OUT), np.float32)
    for b in range(BATCH):
        out[b] = gather_out(res.results[b]["out"])
    return out

